# revision 54
# baseline (speedup 1.0000x reference)
"""Trainium2 Bass kernel for nn_BlockContrastiveLoss (fp8 DoubleRow design).

Math: for x in [B*T, 16, 4], x_hat = x / max(||x||_block, eps) per 4-dim
block. Let q = fp8e4m3(x_hat). The pairwise-cosine sum over each vocab
bin is computed EXACTLY for the quantized vectors via

    sum_{i<j in v} q_i . q_j = (||S_v||^2 - sum_{t in v} ||q_t||^2) / 2

so  numerator = (sum_v ||S_v||^2 - sum_t ||q_t||^2) / 32
    P         = (sum_v C_v^2 - N) / 2          (C = global counts)
    loss      = numerator / max(P, 1) * (P > 0)

The only approximation vs the fp32 reference is q != x_hat (measured
rel. err ~1.3e-2 against the jax oracle, within the 2e-2 gate).

Device strategy (8 cores, data-parallel over B*T):
  - partition-major layout: partition p owns tokens p*256+n, n in 0..256.
  - one-hot as packed fp16 words: word w of k-tile t is 56 (fp8 1.0 in
    low byte) if id==2w, 14336 (high byte) if id==2w+1 -- built in ONE
    DVE tensor_scalar (is_equal, mult) with two per-partition scalar
    pointers (s1 = id>>1, s2 = 56 + 14280*(id&1)). Bitcast to fp8 gives
    the full 1024-wide one-hot for 128 tokens in one ~194ns instruction.
  - fp8 DoubleRow matmuls contract 2 k-tiles (256 tokens) at once at
    0.5 cycles/column. Stationary = [q | 1.0 | 1.0-pad] (80 cols; col 64
    folds the counts row into PSUM row 64; 65..79 pad to satisfy the
    dual-fp8 Ldweights step%16 rule and land in ignored PSUM rows).
    Three matmuls per pair: Gram [80,80] (trace = sum ||q||^2, exact),
    S halves [80,512] x2, accumulated over all 128 pairs.
  - AllReduce [97,1024]: rows 0..63 = S, 64 = counts, [96,0] = sum ||q||^2
    (trace of Gram via identity mask + ones^T matmul). The epilogue
    re-loads the table SPREAD across all 128 partitions (S as [128,512],
    counts as [128,8]) so the final reductions are one wide ACT square
    + one tiny matmul; every core computes the closed form, core 0's
    loss is returned.
"""

import numpy as np

N_CORES = 8
B, T, D = 32, 8192, 64
V = 1024
TOK_PER_CORE = B * T // N_CORES      # 32768
P = 128
NPP = TOK_PER_CORE // P              # 256 tokens per partition (k-tiles)
NPAIR = NPP // 2                     # 128 DoubleRow pairs
NTOK_GLOBAL = float(B * T)

_cache = {}
_opts = {"trace": False}


def _build_nc(single=False):
    import concourse.bacc as bacc
    import concourse.mybir as mybir
    import concourse.tile as tile

    dt = mybir.dt
    f32, f16, i32, i16 = dt.float32, dt.float16, dt.int32, dt.int16
    f8 = dt.float8e4
    AF = mybir.ActivationFunctionType
    OP = mybir.AluOpType
    AX = mybir.AxisListType
    PM = mybir.MatmulPerfMode

    nc = bacc.Bacc("TRN2", target_bir_lowering=False, debug=False,
                   num_devices=1 if single else N_CORES)

    x_dram = nc.dram_tensor("x", [TOK_PER_CORE, D], f32, kind="ExternalInput")
    tok_dram = nc.dram_tensor("tok", [TOK_PER_CORE], i32, kind="ExternalInput")
    loss_dram = nc.dram_tensor("loss", [1, 1], f32, kind="ExternalOutput")
    cc_in = nc.dram_tensor("cc_in", [65, V + 1], f32)
    cc_out = nc.dram_tensor("cc_out", [65, V + 1], f32)

    with tile.TileContext(nc) as tc:
        with (
            tc.tile_pool(name="const", bufs=1) as constp,
            tc.tile_pool(name="xin", bufs=4) as xp,
            tc.tile_pool(name="mid", bufs=5) as midp,
            tc.tile_pool(name="oh", bufs=3) as ohp,
            tc.tile_pool(name="psum", bufs=1, space="PSUM") as psp,
            tc.tile_pool(name="ep", bufs=1) as epp,
        ):
            # ---- constants / preamble ----
            iota_i = constp.tile([P, V // 2], i16)
            nc.gpsimd.iota(iota_i[:], pattern=[[1, V // 2]], base=0,
                           channel_multiplier=0)

            ids_i = constp.tile([P, NPP], i32)
            tok_v = tok_dram.ap().rearrange("(p n) -> p n", p=P)
            s1 = constp.tile([P, NPP], f32)     # floor(id/2)
            s2 = constp.tile([P, NPP], f32)     # 56 + 14280*(id&1)
            s1i = constp.tile([P, NPP], i32)
            pari = constp.tile([P, NPP], i32)
            # sliced so the first pairs' scalars are ready quickly
            for a, b in ((0, 16), (16, 64), (64, NPP)):
                sl = slice(a, b)
                eng = nc.vector if a == 0 else nc.gpsimd
                nc.sync.dma_start(ids_i[:, sl], tok_v[:, sl])
                nc.vector.tensor_scalar(s1i[:, sl], ids_i[:, sl], 1, None,
                                        OP.logical_shift_right)
                nc.vector.tensor_scalar(pari[:, sl], ids_i[:, sl], 1, None,
                                        OP.bitwise_and)
                eng.tensor_copy(s1[:, sl], s1i[:, sl])
                eng.tensor_copy(s2[:, sl], pari[:, sl])
                eng.tensor_scalar(s2[:, sl], s2[:, sl], 14280.0, None,
                                  OP.mult)
                eng.tensor_scalar(s2[:, sl], s2[:, sl], 56.0, None,
                                  OP.add)

            eps2 = constp.tile([P, 1], f32)
            nc.vector.memset(eps2[:], 1e-12)

            # preload ACT tables so chunk 0's chain doesn't pay the
            # 1.3us table load twice on its critical path
            warm = constp.tile([P, 2], f32)
            nc.scalar.activation(warm[:, 0:1], eps2[:], AF.Square)
            nc.scalar.activation(warm[:, 1:2], eps2[:],
                                 AF.Abs_reciprocal_sqrt, bias=eps2[:])

            # persistent x8 chunk buffers (manual rotation): [P, 16, 80] fp8
            # cols 64..79 = 1.0 once (col 64 is the counts row; 65..79 land
            # in ignored PSUM rows); the big-mult touches cols 0..63 only.
            # Buffer 0 is set before the loop; 1 and 2 are emitted after
            # chunk 0 so they don't delay the first matmul.
            NX8 = 5
            x8bufs = []
            for i in range(NX8):
                x8b = constp.tile([P, 16, 80], f8, tag=f"x8b{i}")
                x8bufs.append(x8b)
            nc.gpsimd.memset(x8bufs[0][:, :, 64:80], 1.0)

            # tail constants/staging, emitted mid-loop (see below)
            ioc = constp.tile([64, 64], i16)
            idn = constp.tile([64, 64], f32)
            ones64 = constp.tile([64, 1], f32)
            ones128 = constp.tile([P, 1], f32)


            # PSUM: S rows 0..63 = S, 64 = counts, 65..79 junk
            S_ps = psp.tile([80, V], f32)
            G_ps = psp.tile([80, 80], f32)
            J_ps = psp.tile([80, 512], f32)

            x_n = x_dram.ap().rearrange("(p n) d -> p n d", p=P)

            widths = [2, 2, 4, 8] + [16] * ((NPP - 16) // 16)
            assert sum(widths) == NPP

            # Software-pipelined: the x-side chain (DMA -> sq -> t2 -> n2
            # -> rinv -> fp8 quantize) runs LAG chunks ahead of the
            # one-hot + matmul phase, so the PE bursts are gated by the
            # one-hot stream, not by the quantize chain's serial latency.
            LAG = 2
            state = {"pair_idx": 0}

            def emit_ohmm(W, n0, x8, fill=False):
                oh16 = ohp.tile([P, 16, V // 2], i16, tag="oh16")
                for t in range(W):
                    nc.vector.tensor_scalar(
                        oh16[:, t, :], iota_i[:],
                        s1[:, n0 + t:n0 + t + 1],
                        s2[:, n0 + t:n0 + t + 1],
                        OP.is_equal, OP.mult)
                oh8 = oh16[:].bitcast(f8)  # [P, 16, V]
                for j in range(W // 2):
                    st = x8[:, 2 * j:2 * j + 2, :]
                    first = state["pair_idx"] == 0
                    last = state["pair_idx"] == NPAIR - 1
                    nc.tensor.matmul(G_ps[:], st, st,
                                     start=first, stop=last,
                                     perf_mode=PM.DoubleRow)
                    mv = oh8[:, 2 * j:2 * j + 2, :]
                    nc.tensor.matmul(S_ps[:, 0:512], st, mv[:, :, 0:512],
                                     start=first, stop=last,
                                     perf_mode=PM.DoubleRow)
                    nc.tensor.matmul(S_ps[:, 512:V], st, mv[:, :, 512:V],
                                     start=first, stop=last,
                                     perf_mode=PM.DoubleRow)
                    state["pair_idx"] += 1
                if fill:
                    st = x8[:, W - 2:W, :]
                    mv = oh8[:, W - 2:W, :]
                    for _ in range(14):
                        nc.tensor.matmul(J_ps[:], st, mv[:, :, 0:512],
                                         start=True, stop=True,
                                         perf_mode=PM.DoubleRow)

            def emit_xchain(ci, W, n0, xt):
                xtf = xt.rearrange("p j d -> p (j d)")
                sq = midp.tile([P, 16 * D], f16, tag="sq")
                sq = sq[:, 0:W * D]
                nc.scalar.activation(sq, xtf, AF.Square)

                sq4 = sq.rearrange("p (t q) -> p t q", q=4)
                t2 = midp.tile([P, 16 * 16, 2], f16, tag="t2")
                t2 = t2[:, 0:W * 16, :]
                nc.vector.tensor_tensor(t2, sq4[:, :, 0:2], sq4[:, :, 2:4],
                                        OP.add)
                n2 = midp.tile([P, 16 * 16], f16, tag="n2")
                n2 = n2[:, 0:W * 16]
                nc.gpsimd.tensor_tensor(n2, t2[:, :, 0], t2[:, :, 1], OP.add)

                rinv = midp.tile([P, 16 * 16], f32, tag="rinv")
                rinv = rinv[:, 0:W * 16]
                nc.scalar.activation(rinv, n2, AF.Abs_reciprocal_sqrt,
                                     bias=eps2[:])

                x8 = x8bufs[ci % NX8]
                xt4 = xt.rearrange("p j (b q) -> p j b q", q=4)
                rin4 = rinv.rearrange("p (j b) -> p j b", b=16)
                rin4 = rin4.unsqueeze(3).broadcast_to([P, W, 16, 4])
                x8w = x8[:, 0:W, 0:64].rearrange("p j (b q) -> p j b q", q=4)
                nc.gpsimd.tensor_tensor(x8w, xt4, rin4, OP.mult)
                return x8

            stages = []
            n_off = 0
            for ci, W in enumerate(widths):
                xt = xp.tile([P, 16, D], f32, tag="xt")
                xt = xt[:, 0:W, :]
                nc.sync.dma_start(xt, x_n[:, n_off:n_off + W, :])
                stages.append([W, n_off, xt, None])
                n_off += W
                if ci < LAG:
                    # prologue: x-chain ahead of the first one-hots
                    stages[ci][3] = emit_xchain(ci, W, stages[ci][1], xt)
                else:
                    # one-hot + matmuls for chunk ci-LAG, then this chunk's
                    # quantize chain (so the in-order DVE queue never blocks
                    # on the chain's inputs while one-hot work is ready).
                    # The last two in-loop batches append PE filler to keep
                    # the tensor engine's p-state ramp alive into the drain.
                    emit_ohmm(stages[ci - LAG][0], stages[ci - LAG][1],
                              stages[ci - LAG][3],
                              fill=ci >= len(widths) - 6)
                    stages[ci][3] = emit_xchain(ci, W, stages[ci][1], xt)
                if ci == 0:
                    for bi in range(1, NX8):
                        nc.gpsimd.memset(x8bufs[bi][:, :, 64:80], 1.0)
                if ci == 5:
                    nc.gpsimd.iota(ioc[:], pattern=[[1, 64]], base=0,
                                   channel_multiplier=-1)
                    nc.gpsimd.tensor_scalar(idn[:], ioc[:], 0.0, None,
                                            OP.is_equal)
                    nc.gpsimd.memset(ones64[:], 1.0)
                    nc.gpsimd.memset(ones128[:], 1.0)
            for sstage in stages[len(stages) - LAG:]:
                emit_ohmm(sstage[0], sstage[1], sstage[3])

            # ---- tail ----
            Sc = epp.tile([65, V + 1], f32)
            # sigma = trace of the Gram accumulator, stored in the
            # staging table's extra column (row 64, col 1024) so it rides
            # the half-1 DMA legs
            Gc = epp.tile([64, 64], f32)
            nc.vector.tensor_copy(Gc[:], G_ps[0:64, 0:64])
            gd = epp.tile([64, 64], f32)
            nc.vector.tensor_tensor(gd[:], Gc[:], idn[:], OP.mult)
            gdr = epp.tile([64, 1], f32)
            nc.vector.tensor_reduce(gdr[:], gd[:], AX.X, OP.add)
            sig_ps = psp.tile([1, 1], f32)
            nc.tensor.matmul(sig_ps[:], ones64[:], gdr[:],
                             start=True, stop=True)
            nc.vector.tensor_copy(Sc[64:65, V:V + 1], sig_ps[:])

            for h in (0, 1):
                hs = slice(512 * h, 512 * h + 512 + h)  # half 1 takes col V
                nc.vector.tensor_copy(Sc[:, 512 * h:512 * (h + 1)],
                                      S_ps[0:65, 512 * h:512 * (h + 1)])
                nc.sync.dma_start(cc_in.ap()[0:65, hs], Sc[:, hs])
                if single:
                    nc.sync.dma_start(cc_out.ap()[0:65, hs],
                                      cc_in.ap()[0:65, hs])

            if not single:
                nc.gpsimd.collective_compute(
                    "AllReduce", OP.add,
                    replica_groups=[list(range(N_CORES))],
                    ins=[cc_in.ap().opt()], outs=[cc_out.ap().opt()],
                )

            # ---- epilogue: spread returns so the reductions go wide ----
            Rsp = epp.tile([128, 512], f32)   # S rows spread over 128 parts
            ct = epp.tile([128, 8], f32)      # counts spread over 128 parts
            sgr = epp.tile([1, 1], f32)       # global sum of q^2
            for h in (0, 1):
                hs = slice(512 * h, 512 * (h + 1))
                nc.sync.dma_start(Rsp[:, 256 * h:256 * (h + 1)],
                                  cc_out.ap()[0:64, hs])
            nc.sync.dma_start(ct[:], cc_out.ap()[64:65, 0:V])
            nc.sync.dma_start(sgr[:], cc_out.ap()[64:65, V:V + 1])

            acc2 = epp.tile([128, 3], f32)
            Rsq = epp.tile([128, 512], f16)
            for h in (0, 1):
                cs = slice(256 * h, 256 * (h + 1))
                nc.scalar.activation(Rsq[:, cs], Rsp[:, cs], AF.Square,
                                     accum_out=acc2[:, h:h + 1])
            ctsq = epp.tile([128, 8], f32)
            nc.vector.tensor_tensor(ctsq[:], ct[:], ct[:], OP.mult)
            nc.vector.tensor_reduce(acc2[:, 2:3], ctsq[:], AX.X, OP.add)

            red_ps = psp.tile([1, 3], f32)
            nc.tensor.matmul(red_ps[:], ones128[:], acc2[:],
                             start=True, stop=True)
            red = epp.tile([1, 3], f32)
            nc.vector.tensor_copy(red[:], red_ps[:])

            num = epp.tile([1, 1], f32)
            nc.vector.tensor_tensor(num[:], red[:, 0:1], red[:, 1:2], OP.add)
            nc.vector.tensor_tensor(num[:], num[:], sgr[:], OP.subtract)
            nc.vector.tensor_scalar(num[:], num[:], 1.0 / 32.0, None, OP.mult)

            pm = epp.tile([1, 1], f32)
            nc.vector.tensor_scalar(pm[:], red[:, 2:3], NTOK_GLOBAL, None,
                                    OP.subtract)
            nc.vector.tensor_scalar(pm[:], pm[:], 0.5, None, OP.mult)
            denom = epp.tile([1, 1], f32)
            nc.vector.tensor_scalar(denom[:], pm[:], 1.0, None, OP.max)
            maskp = epp.tile([1, 1], f32)
            nc.vector.tensor_scalar(maskp[:], pm[:], 0.0, None, OP.is_gt)
            rden = epp.tile([1, 1], f32)
            nc.vector.reciprocal(rden[:], denom[:])
            lossv = epp.tile([1, 1], f32)
            nc.vector.tensor_tensor(lossv[:], num[:], rden[:], OP.mult)
            nc.vector.tensor_tensor(lossv[:], lossv[:], maskp[:], OP.mult)
            nc.sync.dma_start(loss_dram.ap(), lossv[:])

    nc.compile()
    return nc


def kernel(semantic_state, token_ids):
    from concourse.bass_utils import run_bass_kernel_spmd

    if "nc" not in _cache:
        _cache["nc"] = _build_nc()
    nc = _cache["nc"]

    x = np.ascontiguousarray(np.asarray(semantic_state, dtype=np.float32)
                             ).reshape(N_CORES, TOK_PER_CORE, D)
    t = np.ascontiguousarray(np.asarray(token_ids).astype(np.int32)
                             ).reshape(N_CORES, TOK_PER_CORE)
    in_maps = [{"x": x[c], "tok": t[c]} for c in range(N_CORES)]
    res = run_bass_kernel_spmd(nc, in_maps, core_ids=list(range(N_CORES)),
                               trace=_opts["trace"])
    _cache["last_res"] = res
    out = np.asarray(res.results[0]["loss"], dtype=np.float32)
    return out.reshape(())


# revision 55
# speedup vs baseline: 1.0045x; 1.0045x over previous
"""Trainium2 Bass kernel for nn_BlockContrastiveLoss (fp8 DoubleRow design).

Math: for x in [B*T, 16, 4], x_hat = x / max(||x||_block, eps) per 4-dim
block. Let q = fp8e4m3(x_hat). The pairwise-cosine sum over each vocab
bin is computed EXACTLY for the quantized vectors via

    sum_{i<j in v} q_i . q_j = (||S_v||^2 - sum_{t in v} ||q_t||^2) / 2

so  numerator = (sum_v ||S_v||^2 - sum_t ||q_t||^2) / 32
    P         = (sum_v C_v^2 - N) / 2          (C = global counts)
    loss      = numerator / max(P, 1) * (P > 0)

The only approximation vs the fp32 reference is q != x_hat (measured
rel. err ~1.3e-2 against the jax oracle, within the 2e-2 gate).

Device strategy (8 cores, data-parallel over B*T):
  - partition-major layout: partition p owns tokens p*256+n, n in 0..256.
  - one-hot as packed fp16 words: word w of k-tile t is 56 (fp8 1.0 in
    low byte) if id==2w, 14336 (high byte) if id==2w+1 -- built in ONE
    DVE tensor_scalar (is_equal, mult) with two per-partition scalar
    pointers (s1 = id>>1, s2 = 56 + 14280*(id&1)). Bitcast to fp8 gives
    the full 1024-wide one-hot for 128 tokens in one ~194ns instruction.
  - fp8 DoubleRow matmuls contract 2 k-tiles (256 tokens) at once at
    0.5 cycles/column. Stationary = [q | 1.0 | 1.0-pad] (80 cols; col 64
    folds the counts row into PSUM row 64; 65..79 pad to satisfy the
    dual-fp8 Ldweights step%16 rule and land in ignored PSUM rows).
    Three matmuls per pair: Gram [80,80] (trace = sum ||q||^2, exact),
    S halves [80,512] x2, accumulated over all 128 pairs.
  - AllReduce [97,1024]: rows 0..63 = S, 64 = counts, [96,0] = sum ||q||^2
    (trace of Gram via identity mask + ones^T matmul). The epilogue
    re-loads the table SPREAD across all 128 partitions (S as [128,512],
    counts as [128,8]) so the final reductions are one wide ACT square
    + one tiny matmul; every core computes the closed form, core 0's
    loss is returned.
"""

import numpy as np

N_CORES = 8
B, T, D = 32, 8192, 64
V = 1024
TOK_PER_CORE = B * T // N_CORES      # 32768
P = 128
NPP = TOK_PER_CORE // P              # 256 tokens per partition (k-tiles)
NPAIR = NPP // 2                     # 128 DoubleRow pairs
NTOK_GLOBAL = float(B * T)

_cache = {}
_opts = {"trace": False}


def _build_nc(single=False):
    import concourse.bacc as bacc
    import concourse.mybir as mybir
    import concourse.tile as tile

    dt = mybir.dt
    f32, f16, i32, i16 = dt.float32, dt.float16, dt.int32, dt.int16
    f8 = dt.float8e4
    AF = mybir.ActivationFunctionType
    OP = mybir.AluOpType
    AX = mybir.AxisListType
    PM = mybir.MatmulPerfMode

    nc = bacc.Bacc("TRN2", target_bir_lowering=False, debug=False,
                   num_devices=1 if single else N_CORES)

    x_dram = nc.dram_tensor("x", [TOK_PER_CORE, D], f32, kind="ExternalInput")
    tok_dram = nc.dram_tensor("tok", [TOK_PER_CORE], i32, kind="ExternalInput")
    loss_dram = nc.dram_tensor("loss", [1, 1], f32, kind="ExternalOutput")
    cc_in = nc.dram_tensor("cc_in", [65, V + 1], f32)
    cc_out = nc.dram_tensor("cc_out", [65, V + 1], f32)

    with tile.TileContext(nc) as tc:
        with (
            tc.tile_pool(name="const", bufs=1) as constp,
            tc.tile_pool(name="xin", bufs=4) as xp,
            tc.tile_pool(name="mid", bufs=5) as midp,
            tc.tile_pool(name="oh", bufs=3) as ohp,
            tc.tile_pool(name="psum", bufs=1, space="PSUM") as psp,
            tc.tile_pool(name="ep", bufs=1) as epp,
        ):
            # ---- constants / preamble ----
            iota_i = constp.tile([P, V // 2], i16)
            nc.gpsimd.iota(iota_i[:], pattern=[[1, V // 2]], base=0,
                           channel_multiplier=0)

            ids_i = constp.tile([P, NPP], i32)
            tok_v = tok_dram.ap().rearrange("(p n) -> p n", p=P)
            s1 = constp.tile([P, NPP], f32)     # floor(id/2)
            s2 = constp.tile([P, NPP], f32)     # 56 + 14280*(id&1)
            s1i = constp.tile([P, NPP], i32)
            pari = constp.tile([P, NPP], i32)
            # sliced so the first pairs' scalars are ready quickly
            for a, b in ((0, 16), (16, 64), (64, NPP)):
                sl = slice(a, b)
                eng = nc.vector if a == 0 else nc.gpsimd
                nc.sync.dma_start(ids_i[:, sl], tok_v[:, sl])
                nc.vector.tensor_scalar(s1i[:, sl], ids_i[:, sl], 1, None,
                                        OP.logical_shift_right)
                nc.vector.tensor_scalar(pari[:, sl], ids_i[:, sl], 1, None,
                                        OP.bitwise_and)
                eng.tensor_copy(s1[:, sl], s1i[:, sl])
                eng.tensor_copy(s2[:, sl], pari[:, sl])
                eng.tensor_scalar(s2[:, sl], s2[:, sl], 14280.0, None,
                                  OP.mult)
                eng.tensor_scalar(s2[:, sl], s2[:, sl], 56.0, None,
                                  OP.add)

            eps2 = constp.tile([P, 1], f32)
            nc.vector.memset(eps2[:], 1e-12)

            # preload ACT tables so chunk 0's chain doesn't pay the
            # 1.3us table load twice on its critical path
            warm = constp.tile([P, 2], f32)
            nc.scalar.activation(warm[:, 0:1], eps2[:], AF.Square)
            nc.scalar.activation(warm[:, 1:2], eps2[:],
                                 AF.Abs_reciprocal_sqrt, bias=eps2[:])

            # persistent x8 chunk buffers (manual rotation): [P, 16, 80] fp8
            # cols 64..79 = 1.0 once (col 64 is the counts row; 65..79 land
            # in ignored PSUM rows); the big-mult touches cols 0..63 only.
            # Buffer 0 is set before the loop; 1 and 2 are emitted after
            # chunk 0 so they don't delay the first matmul.
            NX8 = 5
            x8bufs = []
            for i in range(NX8):
                x8b = constp.tile([P, 16, 80], f8, tag=f"x8b{i}")
                x8bufs.append(x8b)
            nc.gpsimd.memset(x8bufs[0][:, :, 64:80], 1.0)

            # tail constants/staging, emitted mid-loop (see below)
            ioc = constp.tile([64, 64], i16)
            idn = constp.tile([64, 64], f32)
            ones64 = constp.tile([64, 1], f32)
            ones128 = constp.tile([P, 1], f32)


            # PSUM: S rows 0..63 = S, 64 = counts, 65..79 junk
            S_ps = psp.tile([80, V], f32)
            G_ps = psp.tile([80, 80], f32)
            J_ps = psp.tile([80, 512], f32)

            x_n = x_dram.ap().rearrange("(p n) d -> p n d", p=P)

            widths = [2, 2, 4, 8] + [16] * ((NPP - 16) // 16)
            assert sum(widths) == NPP

            # Software-pipelined: the x-side chain (DMA -> sq -> t2 -> n2
            # -> rinv -> fp8 quantize) runs LAG chunks ahead of the
            # one-hot + matmul phase, so the PE bursts are gated by the
            # one-hot stream, not by the quantize chain's serial latency.
            LAG = 2
            state = {"pair_idx": 0}

            def emit_ohmm(W, n0, x8, fill=False):
                oh16 = ohp.tile([P, 16, V // 2], i16, tag="oh16")
                for t in range(W):
                    nc.vector.tensor_scalar(
                        oh16[:, t, :], iota_i[:],
                        s1[:, n0 + t:n0 + t + 1],
                        s2[:, n0 + t:n0 + t + 1],
                        OP.is_equal, OP.mult)
                oh8 = oh16[:].bitcast(f8)  # [P, 16, V]
                for j in range(W // 2):
                    st = x8[:, 2 * j:2 * j + 2, :]
                    first = state["pair_idx"] == 0
                    last = state["pair_idx"] == NPAIR - 1
                    nc.tensor.matmul(G_ps[:], st, st,
                                     start=first, stop=last,
                                     perf_mode=PM.DoubleRow)
                    mv = oh8[:, 2 * j:2 * j + 2, :]
                    nc.tensor.matmul(S_ps[:, 0:512], st, mv[:, :, 0:512],
                                     start=first, stop=last,
                                     perf_mode=PM.DoubleRow)
                    nc.tensor.matmul(S_ps[:, 512:V], st, mv[:, :, 512:V],
                                     start=first, stop=last,
                                     perf_mode=PM.DoubleRow)
                    state["pair_idx"] += 1
                if fill:
                    st = x8[:, W - 2:W, :]
                    mv = oh8[:, W - 2:W, :]
                    for _ in range(16):
                        nc.tensor.matmul(J_ps[:], st, mv[:, :, 0:512],
                                         start=True, stop=True,
                                         perf_mode=PM.DoubleRow)

            def emit_xchain(ci, W, n0, xt):
                xtf = xt.rearrange("p j d -> p (j d)")
                sq = midp.tile([P, 16 * D], f16, tag="sq")
                sq = sq[:, 0:W * D]
                nc.scalar.activation(sq, xtf, AF.Square)

                sq4 = sq.rearrange("p (t q) -> p t q", q=4)
                t2 = midp.tile([P, 16 * 16, 2], f16, tag="t2")
                t2 = t2[:, 0:W * 16, :]
                nc.vector.tensor_tensor(t2, sq4[:, :, 0:2], sq4[:, :, 2:4],
                                        OP.add)
                n2 = midp.tile([P, 16 * 16], f16, tag="n2")
                n2 = n2[:, 0:W * 16]
                nc.gpsimd.tensor_tensor(n2, t2[:, :, 0], t2[:, :, 1], OP.add)

                rinv = midp.tile([P, 16 * 16], f32, tag="rinv")
                rinv = rinv[:, 0:W * 16]
                nc.scalar.activation(rinv, n2, AF.Abs_reciprocal_sqrt,
                                     bias=eps2[:])

                x8 = x8bufs[ci % NX8]
                xt4 = xt.rearrange("p j (b q) -> p j b q", q=4)
                rin4 = rinv.rearrange("p (j b) -> p j b", b=16)
                rin4 = rin4.unsqueeze(3).broadcast_to([P, W, 16, 4])
                x8w = x8[:, 0:W, 0:64].rearrange("p j (b q) -> p j b q", q=4)
                nc.gpsimd.tensor_tensor(x8w, xt4, rin4, OP.mult)
                return x8

            stages = []
            n_off = 0
            for ci, W in enumerate(widths):
                xt = xp.tile([P, 16, D], f32, tag="xt")
                xt = xt[:, 0:W, :]
                nc.sync.dma_start(xt, x_n[:, n_off:n_off + W, :])
                stages.append([W, n_off, xt, None])
                n_off += W
                if ci < LAG:
                    # prologue: x-chain ahead of the first one-hots
                    stages[ci][3] = emit_xchain(ci, W, stages[ci][1], xt)
                else:
                    # one-hot + matmuls for chunk ci-LAG, then this chunk's
                    # quantize chain (so the in-order DVE queue never blocks
                    # on the chain's inputs while one-hot work is ready).
                    # The last two in-loop batches append PE filler to keep
                    # the tensor engine's p-state ramp alive into the drain.
                    emit_ohmm(stages[ci - LAG][0], stages[ci - LAG][1],
                              stages[ci - LAG][3],
                              fill=ci >= len(widths) - 6)
                    stages[ci][3] = emit_xchain(ci, W, stages[ci][1], xt)
                if ci == 0:
                    for bi in range(1, NX8):
                        nc.gpsimd.memset(x8bufs[bi][:, :, 64:80], 1.0)
                if ci == 5:
                    nc.gpsimd.iota(ioc[:], pattern=[[1, 64]], base=0,
                                   channel_multiplier=-1)
                    nc.gpsimd.tensor_scalar(idn[:], ioc[:], 0.0, None,
                                            OP.is_equal)
                    nc.gpsimd.memset(ones64[:], 1.0)
                    nc.gpsimd.memset(ones128[:], 1.0)
            for sstage in stages[len(stages) - LAG:]:
                emit_ohmm(sstage[0], sstage[1], sstage[3])

            # ---- tail ----
            Sc = epp.tile([65, V + 1], f32)
            # sigma = trace of the Gram accumulator, stored in the
            # staging table's extra column (row 64, col 1024) so it rides
            # the half-1 DMA legs
            Gc = epp.tile([64, 64], f32)
            nc.vector.tensor_copy(Gc[:], G_ps[0:64, 0:64])
            gd = epp.tile([64, 64], f32)
            nc.vector.tensor_tensor(gd[:], Gc[:], idn[:], OP.mult)
            gdr = epp.tile([64, 1], f32)
            nc.vector.tensor_reduce(gdr[:], gd[:], AX.X, OP.add)
            sig_ps = psp.tile([1, 1], f32)
            nc.tensor.matmul(sig_ps[:], ones64[:], gdr[:],
                             start=True, stop=True)
            nc.vector.tensor_copy(Sc[64:65, V:V + 1], sig_ps[:])

            for h in (0, 1):
                hs = slice(512 * h, 512 * h + 512 + h)  # half 1 takes col V
                nc.vector.tensor_copy(Sc[:, 512 * h:512 * (h + 1)],
                                      S_ps[0:65, 512 * h:512 * (h + 1)])
                nc.sync.dma_start(cc_in.ap()[0:65, hs], Sc[:, hs])
                if single:
                    nc.sync.dma_start(cc_out.ap()[0:65, hs],
                                      cc_in.ap()[0:65, hs])

            if not single:
                nc.gpsimd.collective_compute(
                    "AllReduce", OP.add,
                    replica_groups=[list(range(N_CORES))],
                    ins=[cc_in.ap().opt()], outs=[cc_out.ap().opt()],
                )

            # ---- epilogue: spread returns so the reductions go wide ----
            Rsp = epp.tile([128, 512], f32)   # S rows spread over 128 parts
            ct = epp.tile([128, 8], f32)      # counts spread over 128 parts
            sgr = epp.tile([1, 1], f32)       # global sum of q^2
            for h in (0, 1):
                hs = slice(512 * h, 512 * (h + 1))
                nc.sync.dma_start(Rsp[:, 256 * h:256 * (h + 1)],
                                  cc_out.ap()[0:64, hs])
            nc.sync.dma_start(ct[:], cc_out.ap()[64:65, 0:V])
            nc.sync.dma_start(sgr[:], cc_out.ap()[64:65, V:V + 1])

            acc2 = epp.tile([128, 3], f32)
            Rsq = epp.tile([128, 512], f16)
            for h in (0, 1):
                cs = slice(256 * h, 256 * (h + 1))
                nc.scalar.activation(Rsq[:, cs], Rsp[:, cs], AF.Square,
                                     accum_out=acc2[:, h:h + 1])
            ctsq = epp.tile([128, 8], f32)
            nc.vector.tensor_tensor(ctsq[:], ct[:], ct[:], OP.mult)
            nc.vector.tensor_reduce(acc2[:, 2:3], ctsq[:], AX.X, OP.add)

            red_ps = psp.tile([1, 3], f32)
            nc.tensor.matmul(red_ps[:], ones128[:], acc2[:],
                             start=True, stop=True)
            red = epp.tile([1, 3], f32)
            nc.vector.tensor_copy(red[:], red_ps[:])

            num = epp.tile([1, 1], f32)
            nc.vector.tensor_tensor(num[:], red[:, 0:1], red[:, 1:2], OP.add)
            nc.vector.tensor_tensor(num[:], num[:], sgr[:], OP.subtract)
            nc.vector.tensor_scalar(num[:], num[:], 1.0 / 32.0, None, OP.mult)

            pm = epp.tile([1, 1], f32)
            nc.vector.tensor_scalar(pm[:], red[:, 2:3], NTOK_GLOBAL, None,
                                    OP.subtract)
            nc.vector.tensor_scalar(pm[:], pm[:], 0.5, None, OP.mult)
            denom = epp.tile([1, 1], f32)
            nc.vector.tensor_scalar(denom[:], pm[:], 1.0, None, OP.max)
            maskp = epp.tile([1, 1], f32)
            nc.vector.tensor_scalar(maskp[:], pm[:], 0.0, None, OP.is_gt)
            rden = epp.tile([1, 1], f32)
            nc.vector.reciprocal(rden[:], denom[:])
            lossv = epp.tile([1, 1], f32)
            nc.vector.tensor_tensor(lossv[:], num[:], rden[:], OP.mult)
            nc.vector.tensor_tensor(lossv[:], lossv[:], maskp[:], OP.mult)
            nc.sync.dma_start(loss_dram.ap(), lossv[:])

    nc.compile()
    return nc


def kernel(semantic_state, token_ids):
    from concourse.bass_utils import run_bass_kernel_spmd

    if "nc" not in _cache:
        _cache["nc"] = _build_nc()
    nc = _cache["nc"]

    x = np.ascontiguousarray(np.asarray(semantic_state, dtype=np.float32)
                             ).reshape(N_CORES, TOK_PER_CORE, D)
    t = np.ascontiguousarray(np.asarray(token_ids).astype(np.int32)
                             ).reshape(N_CORES, TOK_PER_CORE)
    in_maps = [{"x": x[c], "tok": t[c]} for c in range(N_CORES)]
    res = run_bass_kernel_spmd(nc, in_maps, core_ids=list(range(N_CORES)),
                               trace=_opts["trace"])
    _cache["last_res"] = res
    out = np.asarray(res.results[0]["loss"], dtype=np.float32)
    return out.reshape(())
